# revision 44
# baseline (speedup 1.0000x reference)
"""Distributed multi-head attention (RoPE, non-causal) on 8 TRN2 NeuronCores.

Sharding: tensor-parallel over heads. Core c owns heads {2c, 2c+1}:
  - wq/wk/wv rows c*256:(c+1)*256 (output dim), x replicated (pre-transposed),
  - attention computed locally per (batch, head),
  - quarter-batch AllGather of the attention outputs (transposed layout, bf16),
  - each core then computes output columns c*256:(c+1)*256 with its wo rows.
Host side only shards/casts inputs and concatenates the 8 output column
slices -- all FLOPs run on device.

Structure: software-pipelined at quarter-batch (512 q columns) granularity:
attention units (h, qc) stream scores->exp->attn@v; when both heads of a
quarter land in the bounce buffer its AllGather fires. The matching wo stage
is emitted ~4 quarters LATER (tapering to 3 at the end): engine queues are
in-order, so wo matmuls emitted before their AG completes (17-30us incl.
cross-core skew) would head-of-line block the PE queue. A dummy 256B/rank
AllGather at t~0 absorbs cross-core launch skew (up to ~100us observed)
under proj(0) so the first real AG doesn't pay it.

Layout/precision tricks:
  - All matmuls in bf16 (PSUM accumulates f32); rel-err ~5.7e-3.
  - x and all weights are HOST-pre-blocked to [128 partitions, contiguous]
    so every load DMA is 128 descriptors of >=4KB (fast issue + transfer).
  - RoPE pairs (even/odd head-dim) are separated into halves by permuting the
    wq/wk ROWS on the host, so on device RoPE is 4 full-width DVE ops against
    duplicated cos/sin tables. Permutation is applied identically to q and k,
    so q.k is unchanged.
  - Scores are computed transposed ([k, q]) so exp(scores) feeds attn@v as
    lhsT with no transpose; softmax denominator comes from a ones-column
    appended to v (matmul N=129). No max-subtraction: |scores| < ~10 here.
  - exp batched as k-block groups 2,1,2,1,2 per half: pairs land in a 2-bank
    PSUM tile and get one [128,1024] activation (amortizes the ~350-cycle
    ACT fixed cost); the single interleaves so the pair tile drains.
  - attn output normalized per-partition (q) then PE-transposed to [hd, q]
    so the AllGather concatenates cores along the o dim.
  - wo stage computes out.T (N=512 moving dim) in bf16; host transposes back.
DMA queues: sync (HWDGE, otherwise idle) issues input loads, bounce writes,
gather-reads and output stores; scalar issues the secondary startup loads;
gpsimd carries ONLY collective triggers (they block the queue while the CC
stream is busy, so nothing else may sit behind them).
"""

import numpy as np
import ml_dtypes

B, S, D, H = 4, 2048, 2048, 16
HD = 128            # head dim
NCORES = 8
HPC = H // NCORES   # heads per core = 2
OSL = HPC * HD      # per-core o-slice = 256
ROWS = B * S        # 8192 flattened rows
DCH = D // 128      # 16 contraction chunks
SCH = 512           # seq chunk for projections
KB = S // 128       # 16 k-blocks per batch
QC = 512            # q chunk in attention
NQC = S // QC       # 4 (= quarters per batch)

BF16 = ml_dtypes.bfloat16
_NC_CACHE = None


def _build():
    import concourse.bass as bass  # noqa: F401
    import concourse.mybir as mybir
    import concourse.tile as tile
    from concourse import bacc
    from concourse.masks import make_identity

    fp32 = mybir.dt.float32
    bf16 = mybir.dt.bfloat16

    nc = bacc.Bacc(
        "TRN2",
        target_bir_lowering=False,
        debug=False,
        num_devices=NCORES,
    )

    # all loads are host-pre-blocked to [128 partitions, ...contiguous] so
    # each DMA is 128 descriptors of >=4KB instead of 1-2K small ones
    NF = B * (S // SCH) * 2  # x fetch units: (batch, seq-chunk, d-half)
    xq = nc.declare_dram_parameter("xq", [128, NF, DCH // 2, SCH], bf16,
                                   isOutput=False)
    wqT = nc.declare_dram_parameter("wqT", [128, DCH, OSL], bf16,
                                    isOutput=False)
    wkT = nc.declare_dram_parameter("wkT", [128, DCH, OSL], bf16,
                                    isOutput=False)
    wvT = nc.declare_dram_parameter("wvT", [128, DCH, OSL], bf16,
                                    isOutput=False)
    woT = nc.declare_dram_parameter("woT", [128, DCH, OSL], bf16,
                                    isOutput=False)
    cosd = nc.declare_dram_parameter("cosd", [128, S], fp32, isOutput=False)
    sind = nc.declare_dram_parameter("sind", [128, S], fp32, isOutput=False)
    outp = nc.declare_dram_parameter("out", [OSL, ROWS], bf16, isOutput=True)

    inv_sqrt_hd = 1.0 / float(np.sqrt(HD))

    with tile.TileContext(nc) as tc:
        with (
            tc.tile_pool(name="glob", bufs=1) as glob,
            tc.tile_pool(name="dram", bufs=1, space="DRAM") as dram,
            tc.tile_pool(name="qkv", bufs=2) as qkv,
            tc.tile_pool(name="xtp", bufs=4) as xtp,
            tc.tile_pool(name="attp", bufs=4) as attp,
            tc.tile_pool(name="gtp", bufs=3) as gtp,
            tc.tile_pool(name="tmpp", bufs=3) as tmpp,
            tc.tile_pool(name="smalls", bufs=4) as smalls,
            tc.tile_pool(name="otp", bufs=2) as otp,
            tc.tile_pool(name="psA", bufs=2, space="PSUM") as psA,
            tc.tile_pool(name="psB", bufs=3, space="PSUM") as psB,
            tc.tile_pool(name="psCD", bufs=2, space="PSUM") as psCD,
            tc.tile_pool(name="psW", bufs=1, space="PSUM") as psW,
        ):
            wq_sb = glob.tile([128, DCH, OSL], bf16, name="wq_sb")
            wk_sb = glob.tile([128, DCH, OSL], bf16, name="wk_sb")
            wv_sb = glob.tile([128, DCH, OSL], bf16, name="wv_sb")
            wo_sb = glob.tile([128, DCH, OSL], bf16, name="wo_sb")
            cosb = glob.tile([128, S], fp32, name="cosb")
            sinb = glob.tile([128, S], fp32, name="sinb")
            # startup: what the first projection needs, first -- and spread
            # across all three DMA-issue queues (sync/scalar/gpsimd) since
            # each rearranged-load issue occupies its queue for 1-5us.
            # gpsimd is free (only collective triggers) so x00 goes there,
            # BEFORE the warmup collective trigger below.
            nc.sync.dma_start(wq_sb[:], wqT[:, :, :])
            xh00 = []
            for half in range(2):
                xth = xtp.tile([128, DCH // 2, SCH], bf16,
                               name=f"xt0_{half}", tag="xt")
                nc.sync.dma_start(xth[:], xq[:, half, :, :])
                xh00.append(xth)
            nc.scalar.dma_start(cosb[:, 0:SCH], cosd[:, 0:SCH])
            nc.scalar.dma_start(sinb[:, 0:SCH], sind[:, 0:SCH])
            nc.sync.dma_start(wk_sb[:], wkT[:, :, :])
            nc.scalar.dma_start(wv_sb[:], wvT[:, :, :])
            for ch in range(1, S // SCH):
                sl = slice(ch * SCH, (ch + 1) * SCH)
                nc.scalar.dma_start(cosb[:, sl], cosd[:, sl])
                nc.scalar.dma_start(sinb[:, sl], sind[:, sl])

            ident = glob.tile([128, 128], bf16, name="ident")
            make_identity(nc, ident[:])

            # quarter-batch granularity: each 512-col quarter of each batch
            # gathers and projects through wo independently
            NU = B * NQC
            bounce = [dram.tile([OSL, QC], bf16, name=f"bounce{u}")
                      for u in range(NU)]
            gath = [dram.tile([NCORES * OSL, QC], bf16, addr_space="Shared",
                              name=f"gath{u}") for u in range(NU)]

            # Dummy collective issued at t~0: absorbs cross-core launch skew
            # (up to ~100us observed) while proj(0) keeps the PE busy, so the
            # first REAL AllGather doesn't pay the skew. Cheap: 256B/rank.
            warm_in = dram.tile([1, 128], bf16, name="warm_in")
            warm_out = dram.tile([NCORES, 128], bf16, addr_space="Shared",
                                 name="warm_out")
            nc.gpsimd.collective_compute(
                "AllGather",
                mybir.AluOpType.bypass,
                ins=[warm_in.opt()],
                outs=[warm_out.opt()],
                replica_groups=[list(range(NCORES))],
            )

            def fetch_x(b, sc):
                xh = []
                for half in range(2):
                    xth = xtp.tile([128, DCH // 2, SCH], bf16,
                                   name=f"xt{half}", tag="xt")
                    nc.sync.dma_start(
                        xth[:], xq[:, (b * (S // SCH) + sc) * 2 + half, :, :])
                    xh.append(xth)
                return xh

            def proj(b, xh_pre):
                qt = qkv.tile([128, HPC, S], bf16, name="qt", tag="qt")
                kt = qkv.tile([128, HPC, S], bf16, name="kt", tag="kt")
                vt = qkv.tile([128, KB, HPC, HD + 1], bf16, name="vt",
                              tag="vt")
                nc.vector.memset(vt[:, :, :, HD:HD + 1], 1.0)
                for sc in range(S // SCH):
                    xh = xh_pre if sc == 0 else fetch_x(b, sc)
                    cosr = cosb[:, sc * SCH:(sc + 1) * SCH]
                    sinr = sinb[:, sc * SCH:(sc + 1) * SCH]
                    for (w_sb, dstT) in ((wq_sb, qt), (wk_sb, kt)):
                        for h in range(HPC):
                            ps = psA.tile([128, SCH], fp32, name="ps_proj",
                                          tag="psA")
                            for c in range(DCH):
                                nc.tensor.matmul(
                                    ps[:],
                                    w_sb[:, c, h * HD:(h + 1) * HD],
                                    xh[c // 8][:, c % 8, :],
                                    start=(c == 0), stop=(c == DCH - 1))
                            m1 = tmpp.tile([128, SCH], fp32, name="m1",
                                           tag="t")
                            m2 = tmpp.tile([128, SCH], fp32, name="m2",
                                           tag="t")
                            # m1 = [tr*cos ; ti*cos]; m2 swapped-halves =
                            # [ti*sin ; tr*sin] so later DVE ops use equal
                            # SBUF base partitions (PSUM operand may differ).
                            nc.vector.tensor_mul(m1[:], ps[:], cosr)
                            nc.vector.tensor_mul(
                                m2[0:64, :], ps[64:128, :], sinr[0:64, :])
                            nc.vector.tensor_mul(
                                m2[64:128, :], ps[0:64, :], sinr[64:128, :])
                            sl = slice(sc * SCH, (sc + 1) * SCH)
                            nc.vector.tensor_sub(
                                dstT[0:64, h, sl], m1[0:64, :], m2[0:64, :])
                            nc.vector.tensor_add(
                                dstT[64:128, h, sl], m2[64:128, :],
                                m1[64:128, :])
                    for ssb in range(SCH // 128):
                        kb = sc * (SCH // 128) + ssb
                        psv = psA.tile([128, OSL], fp32, name="psv", tag="psA")
                        for c in range(DCH):
                            nc.tensor.matmul(
                                psv[:],
                                xh[c // 8][:, c % 8, ssb * 128:(ssb + 1) * 128],
                                wv_sb[:, c, :],
                                start=(c == 0), stop=(c == DCH - 1))
                        nc.vector.tensor_copy(
                            vt[:, kb, :, 0:HD],
                            psv[:].rearrange("p (h d) -> p h d", h=HPC))
                return qt, kt, vt

            def attn_scores(qt, kt, h, qc):
                # k-blocks grouped 2,1,2,1,2 per half: pairs go to a 2-bank
                # PSUM tile and get ONE [128,1024] exp activation (amortizes
                # the ~350-cycle ACT fixed cost); singles interleave so the
                # pair tile's ACT drains while the single's matmul runs.
                halves = []
                for eh in range(2):
                    expT = attp.tile([128, KB // 2, QC], bf16, name="expT",
                                     tag="expT")
                    groups = [(0, 2), (2, 1), (3, 2), (5, 1), (6, 2)]
                    for (j0, glen) in groups:
                        if glen == 2:
                            pss = psB.tile([128, 2, QC], fp32, name="pssp",
                                           tag="psBp", bufs=1)
                        else:
                            pss = psB.tile([128, 1, QC], fp32, name="psss",
                                           tag="psBs", bufs=1)
                        for g in range(glen):
                            kb = eh * (KB // 2) + j0 + g
                            nc.tensor.matmul(
                                pss[:, g, :],
                                kt[:, h, kb * 128:(kb + 1) * 128],
                                qt[:, h, qc * QC:(qc + 1) * QC],
                                start=True, stop=True)
                        nc.scalar.activation(
                            expT[:, j0:j0 + glen, :], pss[:],
                            mybir.ActivationFunctionType.Exp,
                            scale=inv_sqrt_hd)
                    halves.append(expT)
                return halves

            def attn_v(vt, expT, b, h, qc):
                u = NQC * b + qc
                a_t = smalls.tile([128, QC], bf16, name="a_t", tag="a_t")
                for qsb in range(QC // 128):
                    pso = psCD.tile([128, HD + 1], fp32, name="pso",
                                    tag="psCD")
                    for kb in range(KB):
                        nc.tensor.matmul(
                            pso[:],
                            expT[kb // (KB // 2)][:, kb % (KB // 2),
                                                  qsb * 128:(qsb + 1) * 128],
                            vt[:, kb, h, :],
                            start=(kb == 0), stop=(kb == KB - 1))
                    rc = smalls.tile([128, 1], fp32, name="rc", tag="rc")
                    nc.vector.reciprocal(rc[:], pso[:, HD:HD + 1])
                    a_sb = smalls.tile([128, HD], bf16, name="a_sb",
                                       tag="a_sb")
                    nc.vector.tensor_scalar_mul(a_sb[:], pso[:, 0:HD], rc[:])
                    pst = psCD.tile([128, 128], bf16, name="pst", tag="psCD")
                    nc.tensor.transpose(pst[:], a_sb[:], ident[:])
                    nc.vector.tensor_copy(
                        a_t[:, qsb * 128:(qsb + 1) * 128], pst[:])
                # bounce writes go on the sync queue: the gpsimd queue holds
                # the collective triggers, which block it while the CC stream
                # is busy -- a_t writes must not queue behind them.
                nc.sync.dma_start(
                    bounce[u][h * HD:(h + 1) * HD, :], a_t[:])

            def attention(b, qt, kt, vt, post_q, post_h0=None):
                units = [(h, qc) for qc in range(NQC) for h in range(HPC)]
                pend = []
                done = [0]

                def flush_one():
                    eT, ph, pqc = pend.pop(0)
                    attn_v(vt, eT, b, ph, pqc)
                    done[0] += 1
                    if done[0] % 2 == 0:
                        post_q(done[0] // 2 - 1)
                    elif post_h0 is not None:
                        post_h0(done[0] // 2)

                for (h, qc) in units:
                    expT = attn_scores(qt, kt, h, qc)
                    pend.append((expT, h, qc))
                    if len(pend) > 1:
                        flush_one()
                flush_one()

            def allgather(ins_t, outs_t):
                nc.gpsimd.collective_compute(
                    "AllGather",
                    mybir.AluOpType.bypass,
                    ins=[ins_t.opt()],
                    outs=[outs_t.opt()],
                    replica_groups=[list(range(NCORES))],
                )

            def wo_q(b, qtr):
                u = NQC * b + qtr
                col0 = b * S + qtr * QC
                last = u == NU - 1
                gh = []
                for dh in range(2):
                    g = gtp.tile([128, DCH // 2, QC], bf16,
                                 name=f"gt{dh}", tag="gt")
                    # sync queue: keeps these issues out of the scalar
                    # queue's exp-activation stream
                    nc.sync.dma_start(
                        g[:],
                        gath[u][dh * 1024:(dh + 1) * 1024, :]
                        .rearrange("(c p) n -> p c n", p=128))
                    gh.append(g)
                for oc in range(OSL // 128):
                    # the final quarter runs when proj is long done, so it
                    # borrows the 2-bank psA pool: oc1's matmuls don't wait
                    # on oc0's PSUM copy-out
                    pool, tg = (psA, "psA") if last else (psW, "psW")
                    psw = pool.tile([128, QC], fp32, name="psw", tag=tg)
                    for c in range(DCH):
                        nc.tensor.matmul(
                            psw[:],
                            wo_sb[:, c, oc * 128:(oc + 1) * 128],
                            gh[c // 8][:, c % 8, :],
                            start=(c == 0), stop=(c == DCH - 1))
                    out_t = otp.tile([128, QC], bf16, name="out_t",
                                     tag="out_t")
                    nc.vector.tensor_copy(out_t[:], psw[:])
                    nc.sync.dma_start(
                        outp[oc * 128:(oc + 1) * 128, col0:col0 + QC],
                        out_t[:])

            wo_loaded = [False]
            # wo stages lag their AllGather by ~4 quarters so the AG (25-35us
            # with cross-core skew; the first one is slowest) completes before
            # the wo matmuls reach the in-order PE queue -- otherwise they
            # head-of-line block it. Near the end the lag tapers to 2 so only
            # the last two quarters remain to cover the final AG's latency.
            wo_pend = []
            post_idx = [0]
            # pend-size ceiling after each of the 16 posts
            pend_max = [9, 9, 9, 9, 4, 4, 4, 4, 4, 4, 4, 4, 4, 3, 3, 3]

            def make_post(b):
                def cb(qtr):
                    u = NQC * b + qtr
                    allgather(bounce[u], gath[u])
                    if not wo_loaded[0]:
                        nc.sync.dma_start(wo_sb[:], woT[:, :, :])
                        wo_loaded[0] = True
                    wo_pend.append((b, qtr))
                    while len(wo_pend) > pend_max[post_idx[0]]:
                        wo_q(*wo_pend.pop(0))
                    post_idx[0] += 1
                return cb

            xh_pre = xh00
            for b in range(B):
                qt, kt, vt = proj(b, xh_pre)
                if b + 1 < B:
                    xh_pre = fetch_x(b + 1, 0)
                attention(b, qt, kt, vt, make_post(b))
            while wo_pend:
                wo_q(*wo_pend.pop(0))

    nc.compile()
    return nc


def _block_w(wT):
    # [D, OSL] -> [128, DCH, OSL]: partition-major blocks of the contraction
    return np.ascontiguousarray(
        wT.reshape(DCH, 128, OSL).transpose(1, 0, 2)).astype(BF16)


def _shard_inputs(x, freqs_cos, freqs_sin, wq, wk, wv, wo):
    # x blocked to [128, (b, sc, half), DCH/2, SCH] so each device fetch is
    # one contiguous 8KB run per partition
    xb = np.asarray(x, dtype=np.float32).reshape(B, 4, SCH, 2, 8, 128)
    xq = np.ascontiguousarray(xb.transpose(5, 0, 1, 3, 4, 2)
                              .reshape(128, B * 8, 8, SCH)).astype(BF16)
    fcT = np.asarray(freqs_cos, dtype=np.float32).T  # [64, S]
    fsT = np.asarray(freqs_sin, dtype=np.float32).T
    cosd = np.ascontiguousarray(np.concatenate([fcT, fcT], 0))  # [128, S]
    sind = np.ascontiguousarray(np.concatenate([fsT, fsT], 0))
    # even indices (real half) then odd (imag half), per head
    perm = np.concatenate([np.arange(0, HD, 2), np.arange(1, HD, 2)])
    in_maps = []
    for c in range(NCORES):
        rows = slice(c * OSL, (c + 1) * OSL)
        wq_c = np.asarray(wq)[rows].reshape(HPC, HD, D)[:, perm, :].reshape(OSL, D)
        wk_c = np.asarray(wk)[rows].reshape(HPC, HD, D)[:, perm, :].reshape(OSL, D)
        in_maps.append({
            "xq": xq,
            "wqT": _block_w(np.ascontiguousarray(wq_c.T)),
            "wkT": _block_w(np.ascontiguousarray(wk_c.T)),
            "wvT": _block_w(np.ascontiguousarray(np.asarray(wv)[rows].T)),
            "woT": _block_w(np.ascontiguousarray(np.asarray(wo)[rows].T)),
            "cosd": cosd,
            "sind": sind,
        })
    return in_maps


def run(inputs, trace=False, trace_cores=None):
    """Build (cached), run on 8 cores; returns (full_output, BassKernelResults)."""
    global _NC_CACHE
    from concourse.bass_utils import run_bass_kernel_spmd
    if _NC_CACHE is None:
        _NC_CACHE = _build()
    in_maps = _shard_inputs(**inputs)
    res = run_bass_kernel_spmd(
        _NC_CACHE, in_maps, core_ids=list(range(NCORES)), trace=trace,
        trace_cores=trace_cores)
    parts = [np.ascontiguousarray(
        np.asarray(res.results[c]["out"]).astype(np.float32).T)
        for c in range(NCORES)]
    full = np.concatenate(parts, axis=1).reshape(B, S, D)
    return full, res


def kernel(x, freqs_cos, freqs_sin, wq, wk, wv, wo):
    full, _ = run(dict(x=x, freqs_cos=freqs_cos, freqs_sin=freqs_sin,
                       wq=wq, wk=wk, wv=wv, wo=wo))
    return full


# revision 45
# speedup vs baseline: 1.0207x; 1.0207x over previous
"""Distributed multi-head attention (RoPE, non-causal) on 8 TRN2 NeuronCores.

Sharding: tensor-parallel over heads. Core c owns heads {2c, 2c+1}:
  - wq/wk/wv rows c*256:(c+1)*256 (output dim), x replicated (pre-transposed),
  - attention computed locally per (batch, head),
  - quarter-batch AllGather of the attention outputs (transposed layout, bf16),
  - each core then computes output columns c*256:(c+1)*256 with its wo rows.
Host side only shards/casts inputs and concatenates the 8 output column
slices -- all FLOPs run on device.

Structure: software-pipelined at quarter-batch (512 q columns) granularity:
attention units (h, qc) stream scores->exp->attn@v; when both heads of a
quarter land in the bounce buffer its AllGather fires. The matching wo stage
is emitted ~4 quarters LATER (tapering to 3 at the end): engine queues are
in-order, so wo matmuls emitted before their AG completes (17-30us incl.
cross-core skew) would head-of-line block the PE queue. A dummy 256B/rank
AllGather at t~0 absorbs cross-core launch skew (up to ~100us observed)
under proj(0) so the first real AG doesn't pay it.

Layout/precision tricks:
  - All matmuls in bf16 (PSUM accumulates f32); rel-err ~5.7e-3.
  - x and all weights are HOST-pre-blocked to [128 partitions, contiguous]
    so every load DMA is 128 descriptors of >=4KB (fast issue + transfer).
  - RoPE pairs (even/odd head-dim) are separated into halves by permuting the
    wq/wk ROWS on the host, so on device RoPE is 4 full-width DVE ops against
    duplicated cos/sin tables. Permutation is applied identically to q and k,
    so q.k is unchanged.
  - Scores are computed transposed ([k, q]) so exp(scores) feeds attn@v as
    lhsT with no transpose; softmax denominator comes from a ones-column
    appended to v (matmul N=129). No max-subtraction: |scores| < ~10 here.
  - exp batched as k-block groups 2,1,2,1,2 per half: pairs land in a 2-bank
    PSUM tile and get one [128,1024] activation (amortizes the ~350-cycle
    ACT fixed cost); the single interleaves so the pair tile drains.
  - attn output normalized per-partition (q) then PE-transposed to [hd, q]
    so the AllGather concatenates cores along the o dim.
  - wo stage computes out.T (N=512 moving dim) in bf16; host transposes back.
DMA queues: sync (HWDGE, otherwise idle) issues input loads, bounce writes,
gather-reads and output stores; scalar issues the secondary startup loads;
gpsimd carries ONLY collective triggers (they block the queue while the CC
stream is busy, so nothing else may sit behind them).
"""

import numpy as np
import ml_dtypes

B, S, D, H = 4, 2048, 2048, 16
HD = 128            # head dim
NCORES = 8
HPC = H // NCORES   # heads per core = 2
OSL = HPC * HD      # per-core o-slice = 256
ROWS = B * S        # 8192 flattened rows
DCH = D // 128      # 16 contraction chunks
SCH = 512           # seq chunk for projections
KB = S // 128       # 16 k-blocks per batch
QC = 512            # q chunk in attention
NQC = S // QC       # 4 (= quarters per batch)

BF16 = ml_dtypes.bfloat16
_NC_CACHE = None


def _build():
    import concourse.bass as bass  # noqa: F401
    import concourse.mybir as mybir
    import concourse.tile as tile
    from concourse import bacc
    from concourse.masks import make_identity

    fp32 = mybir.dt.float32
    bf16 = mybir.dt.bfloat16

    nc = bacc.Bacc(
        "TRN2",
        target_bir_lowering=False,
        debug=False,
        num_devices=NCORES,
    )

    # all loads are host-pre-blocked to [128 partitions, ...contiguous] so
    # each DMA is 128 descriptors of >=4KB instead of 1-2K small ones
    NF = B * (S // SCH) * 2  # x fetch units: (batch, seq-chunk, d-half)
    xq = nc.declare_dram_parameter("xq", [128, NF, DCH // 2, SCH], bf16,
                                   isOutput=False)
    wqT = nc.declare_dram_parameter("wqT", [128, DCH, OSL], bf16,
                                    isOutput=False)
    wkT = nc.declare_dram_parameter("wkT", [128, DCH, OSL], bf16,
                                    isOutput=False)
    wvT = nc.declare_dram_parameter("wvT", [128, DCH, OSL], bf16,
                                    isOutput=False)
    woT = nc.declare_dram_parameter("woT", [128, DCH, OSL], bf16,
                                    isOutput=False)
    cosd = nc.declare_dram_parameter("cosd", [128, S], bf16, isOutput=False)
    sind = nc.declare_dram_parameter("sind", [128, S], bf16, isOutput=False)
    outp = nc.declare_dram_parameter("out", [OSL, ROWS], bf16, isOutput=True)

    inv_sqrt_hd = 1.0 / float(np.sqrt(HD))

    with tile.TileContext(nc) as tc:
        with (
            tc.tile_pool(name="glob", bufs=1) as glob,
            tc.tile_pool(name="dram", bufs=1, space="DRAM") as dram,
            tc.tile_pool(name="qkv", bufs=2) as qkv,
            tc.tile_pool(name="xtp", bufs=4) as xtp,
            tc.tile_pool(name="attp", bufs=4) as attp,
            tc.tile_pool(name="gtp", bufs=3) as gtp,
            tc.tile_pool(name="tmpp", bufs=3) as tmpp,
            tc.tile_pool(name="smalls", bufs=4) as smalls,
            tc.tile_pool(name="otp", bufs=2) as otp,
            tc.tile_pool(name="psA", bufs=2, space="PSUM") as psA,
            tc.tile_pool(name="psB", bufs=3, space="PSUM") as psB,
            tc.tile_pool(name="psCD", bufs=2, space="PSUM") as psCD,
            tc.tile_pool(name="psW", bufs=1, space="PSUM") as psW,
        ):
            wq_sb = glob.tile([128, DCH, OSL], bf16, name="wq_sb")
            wk_sb = glob.tile([128, DCH, OSL], bf16, name="wk_sb")
            wv_sb = glob.tile([128, DCH, OSL], bf16, name="wv_sb")
            wo_sb = glob.tile([128, DCH, OSL], bf16, name="wo_sb")
            cosb = glob.tile([128, S], bf16, name="cosb")
            sinb = glob.tile([128, S], bf16, name="sinb")
            # startup: what the first projection needs, first -- and spread
            # across all three DMA-issue queues (sync/scalar/gpsimd) since
            # each rearranged-load issue occupies its queue for 1-5us.
            # gpsimd is free (only collective triggers) so x00 goes there,
            # BEFORE the warmup collective trigger below.
            nc.sync.dma_start(wq_sb[:], wqT[:, :, :])
            xh00 = []
            for half in range(2):
                xth = xtp.tile([128, DCH // 2, SCH], bf16,
                               name=f"xt0_{half}", tag="xt")
                nc.sync.dma_start(xth[:], xq[:, half, :, :])
                xh00.append(xth)
            nc.scalar.dma_start(cosb[:, 0:SCH], cosd[:, 0:SCH])
            nc.scalar.dma_start(sinb[:, 0:SCH], sind[:, 0:SCH])
            nc.sync.dma_start(wk_sb[:], wkT[:, :, :])
            nc.scalar.dma_start(wv_sb[:], wvT[:, :, :])
            for ch in range(1, S // SCH):
                sl = slice(ch * SCH, (ch + 1) * SCH)
                nc.scalar.dma_start(cosb[:, sl], cosd[:, sl])
                nc.scalar.dma_start(sinb[:, sl], sind[:, sl])

            ident = glob.tile([128, 128], bf16, name="ident")
            make_identity(nc, ident[:])

            # quarter-batch granularity: each 512-col quarter of each batch
            # gathers and projects through wo independently
            NU = B * NQC
            bounce = [dram.tile([OSL, QC], bf16, name=f"bounce{u}")
                      for u in range(NU)]
            gath = [dram.tile([NCORES * OSL, QC], bf16, addr_space="Shared",
                              name=f"gath{u}") for u in range(NU)]

            # Dummy collective issued at t~0: absorbs cross-core launch skew
            # (up to ~100us observed) while proj(0) keeps the PE busy, so the
            # first REAL AllGather doesn't pay the skew. Cheap: 256B/rank.
            warm_in = dram.tile([1, 128], bf16, name="warm_in")
            warm_out = dram.tile([NCORES, 128], bf16, addr_space="Shared",
                                 name="warm_out")
            nc.gpsimd.collective_compute(
                "AllGather",
                mybir.AluOpType.bypass,
                ins=[warm_in.opt()],
                outs=[warm_out.opt()],
                replica_groups=[list(range(NCORES))],
            )

            def fetch_x(b, sc):
                xh = []
                for half in range(2):
                    xth = xtp.tile([128, DCH // 2, SCH], bf16,
                                   name=f"xt{half}", tag="xt")
                    nc.sync.dma_start(
                        xth[:], xq[:, (b * (S // SCH) + sc) * 2 + half, :, :])
                    xh.append(xth)
                return xh

            def proj(b, xh_pre):
                qt = qkv.tile([128, HPC, S], bf16, name="qt", tag="qt")
                kt = qkv.tile([128, HPC, S], bf16, name="kt", tag="kt")
                vt = qkv.tile([128, KB, HPC, HD + 1], bf16, name="vt",
                              tag="vt")
                nc.vector.memset(vt[:, :, :, HD:HD + 1], 1.0)
                for sc in range(S // SCH):
                    xh = xh_pre if sc == 0 else fetch_x(b, sc)
                    cosr = cosb[:, sc * SCH:(sc + 1) * SCH]
                    sinr = sinb[:, sc * SCH:(sc + 1) * SCH]
                    for (w_sb, dstT) in ((wq_sb, qt), (wk_sb, kt)):
                        for h in range(HPC):
                            ps = psA.tile([128, SCH], fp32, name="ps_proj",
                                          tag="psA")
                            for c in range(DCH):
                                nc.tensor.matmul(
                                    ps[:],
                                    w_sb[:, c, h * HD:(h + 1) * HD],
                                    xh[c // 8][:, c % 8, :],
                                    start=(c == 0), stop=(c == DCH - 1))
                            m1 = tmpp.tile([128, SCH], fp32, name="m1",
                                           tag="t")
                            m2 = tmpp.tile([128, SCH], fp32, name="m2",
                                           tag="t")
                            # m1 = [tr*cos ; ti*cos]; m2 swapped-halves =
                            # [ti*sin ; tr*sin] so later DVE ops use equal
                            # SBUF base partitions (PSUM operand may differ).
                            nc.vector.tensor_mul(m1[:], ps[:], cosr)
                            nc.vector.tensor_mul(
                                m2[0:64, :], ps[64:128, :], sinr[0:64, :])
                            nc.vector.tensor_mul(
                                m2[64:128, :], ps[0:64, :], sinr[64:128, :])
                            sl = slice(sc * SCH, (sc + 1) * SCH)
                            nc.vector.tensor_sub(
                                dstT[0:64, h, sl], m1[0:64, :], m2[0:64, :])
                            nc.vector.tensor_add(
                                dstT[64:128, h, sl], m2[64:128, :],
                                m1[64:128, :])
                    for ssb in range(SCH // 128):
                        kb = sc * (SCH // 128) + ssb
                        psv = psA.tile([128, OSL], fp32, name="psv", tag="psA")
                        for c in range(DCH):
                            nc.tensor.matmul(
                                psv[:],
                                xh[c // 8][:, c % 8, ssb * 128:(ssb + 1) * 128],
                                wv_sb[:, c, :],
                                start=(c == 0), stop=(c == DCH - 1))
                        nc.vector.tensor_copy(
                            vt[:, kb, :, 0:HD],
                            psv[:].rearrange("p (h d) -> p h d", h=HPC))
                return qt, kt, vt

            def attn_scores(qt, kt, h, qc):
                # k-blocks grouped 2,1,2,1,2 per half: pairs go to a 2-bank
                # PSUM tile and get ONE [128,1024] exp activation (amortizes
                # the ~350-cycle ACT fixed cost); singles interleave so the
                # pair tile's ACT drains while the single's matmul runs.
                halves = []
                for eh in range(2):
                    expT = attp.tile([128, KB // 2, QC], bf16, name="expT",
                                     tag="expT")
                    groups = [(0, 2), (2, 1), (3, 2), (5, 1), (6, 2)]
                    for (j0, glen) in groups:
                        if glen == 2:
                            pss = psB.tile([128, 2, QC], fp32, name="pssp",
                                           tag="psBp", bufs=1)
                        else:
                            pss = psB.tile([128, 1, QC], fp32, name="psss",
                                           tag="psBs", bufs=1)
                        for g in range(glen):
                            kb = eh * (KB // 2) + j0 + g
                            nc.tensor.matmul(
                                pss[:, g, :],
                                kt[:, h, kb * 128:(kb + 1) * 128],
                                qt[:, h, qc * QC:(qc + 1) * QC],
                                start=True, stop=True)
                        nc.scalar.activation(
                            expT[:, j0:j0 + glen, :], pss[:],
                            mybir.ActivationFunctionType.Exp,
                            scale=inv_sqrt_hd)
                    halves.append(expT)
                return halves

            def attn_v(vt, expT, b, h, qc):
                u = NQC * b + qc
                a_t = smalls.tile([128, QC], bf16, name="a_t", tag="a_t")
                for qsb in range(QC // 128):
                    pso = psCD.tile([128, HD + 1], fp32, name="pso",
                                    tag="psCD")
                    for kb in range(KB):
                        nc.tensor.matmul(
                            pso[:],
                            expT[kb // (KB // 2)][:, kb % (KB // 2),
                                                  qsb * 128:(qsb + 1) * 128],
                            vt[:, kb, h, :],
                            start=(kb == 0), stop=(kb == KB - 1))
                    rc = smalls.tile([128, 1], fp32, name="rc", tag="rc")
                    nc.vector.reciprocal(rc[:], pso[:, HD:HD + 1])
                    a_sb = smalls.tile([128, HD], bf16, name="a_sb",
                                       tag="a_sb")
                    nc.vector.tensor_scalar_mul(a_sb[:], pso[:, 0:HD], rc[:])
                    pst = psCD.tile([128, 128], bf16, name="pst", tag="psCD")
                    nc.tensor.transpose(pst[:], a_sb[:], ident[:])
                    nc.vector.tensor_copy(
                        a_t[:, qsb * 128:(qsb + 1) * 128], pst[:])
                # bounce writes go on the sync queue: the gpsimd queue holds
                # the collective triggers, which block it while the CC stream
                # is busy -- a_t writes must not queue behind them.
                nc.sync.dma_start(
                    bounce[u][h * HD:(h + 1) * HD, :], a_t[:])

            def attention(b, qt, kt, vt, post_q, post_h0=None):
                units = [(h, qc) for qc in range(NQC) for h in range(HPC)]
                pend = []
                done = [0]

                def flush_one():
                    eT, ph, pqc = pend.pop(0)
                    attn_v(vt, eT, b, ph, pqc)
                    done[0] += 1
                    if done[0] % 2 == 0:
                        post_q(done[0] // 2 - 1)
                    elif post_h0 is not None:
                        post_h0(done[0] // 2)

                for (h, qc) in units:
                    expT = attn_scores(qt, kt, h, qc)
                    pend.append((expT, h, qc))
                    if len(pend) > 1:
                        flush_one()
                flush_one()

            def allgather(ins_t, outs_t):
                nc.gpsimd.collective_compute(
                    "AllGather",
                    mybir.AluOpType.bypass,
                    ins=[ins_t.opt()],
                    outs=[outs_t.opt()],
                    replica_groups=[list(range(NCORES))],
                )

            def wo_q(b, qtr):
                u = NQC * b + qtr
                col0 = b * S + qtr * QC
                last = u == NU - 1
                gh = []
                for dh in range(2):
                    g = gtp.tile([128, DCH // 2, QC], bf16,
                                 name=f"gt{dh}", tag="gt")
                    # sync queue: keeps these issues out of the scalar
                    # queue's exp-activation stream
                    nc.sync.dma_start(
                        g[:],
                        gath[u][dh * 1024:(dh + 1) * 1024, :]
                        .rearrange("(c p) n -> p c n", p=128))
                    gh.append(g)
                for oc in range(OSL // 128):
                    # the final quarter runs when proj is long done, so it
                    # borrows the 2-bank psA pool: oc1's matmuls don't wait
                    # on oc0's PSUM copy-out
                    pool, tg = (psA, "psA") if last else (psW, "psW")
                    psw = pool.tile([128, QC], fp32, name="psw", tag=tg)
                    for c in range(DCH):
                        nc.tensor.matmul(
                            psw[:],
                            wo_sb[:, c, oc * 128:(oc + 1) * 128],
                            gh[c // 8][:, c % 8, :],
                            start=(c == 0), stop=(c == DCH - 1))
                    out_t = otp.tile([128, QC], bf16, name="out_t",
                                     tag="out_t")
                    nc.vector.tensor_copy(out_t[:], psw[:])
                    nc.sync.dma_start(
                        outp[oc * 128:(oc + 1) * 128, col0:col0 + QC],
                        out_t[:])

            wo_loaded = [False]
            # wo stages lag their AllGather by ~4 quarters so the AG (25-35us
            # with cross-core skew; the first one is slowest) completes before
            # the wo matmuls reach the in-order PE queue -- otherwise they
            # head-of-line block it. Near the end the lag tapers to 2 so only
            # the last two quarters remain to cover the final AG's latency.
            wo_pend = []
            post_idx = [0]
            # pend-size ceiling after each of the 16 posts
            pend_max = [9, 9, 9, 9, 4, 4, 4, 4, 4, 4, 4, 4, 4, 3, 3, 3]

            def make_post(b):
                def cb(qtr):
                    u = NQC * b + qtr
                    allgather(bounce[u], gath[u])
                    if not wo_loaded[0]:
                        nc.sync.dma_start(wo_sb[:], woT[:, :, :])
                        wo_loaded[0] = True
                    wo_pend.append((b, qtr))
                    while len(wo_pend) > pend_max[post_idx[0]]:
                        wo_q(*wo_pend.pop(0))
                    post_idx[0] += 1
                return cb

            xh_pre = xh00
            for b in range(B):
                qt, kt, vt = proj(b, xh_pre)
                if b + 1 < B:
                    xh_pre = fetch_x(b + 1, 0)
                attention(b, qt, kt, vt, make_post(b))
            while wo_pend:
                wo_q(*wo_pend.pop(0))

    nc.compile()
    return nc


def _block_w(wT):
    # [D, OSL] -> [128, DCH, OSL]: partition-major blocks of the contraction
    return np.ascontiguousarray(
        wT.reshape(DCH, 128, OSL).transpose(1, 0, 2)).astype(BF16)


def _shard_inputs(x, freqs_cos, freqs_sin, wq, wk, wv, wo):
    # x blocked to [128, (b, sc, half), DCH/2, SCH] so each device fetch is
    # one contiguous 8KB run per partition
    xb = np.asarray(x, dtype=np.float32).reshape(B, 4, SCH, 2, 8, 128)
    xq = np.ascontiguousarray(xb.transpose(5, 0, 1, 3, 4, 2)
                              .reshape(128, B * 8, 8, SCH)).astype(BF16)
    fcT = np.asarray(freqs_cos, dtype=np.float32).T  # [64, S]
    fsT = np.asarray(freqs_sin, dtype=np.float32).T
    cosd = np.ascontiguousarray(np.concatenate([fcT, fcT], 0)).astype(BF16)  # [128, S]
    sind = np.ascontiguousarray(np.concatenate([fsT, fsT], 0)).astype(BF16)
    # even indices (real half) then odd (imag half), per head
    perm = np.concatenate([np.arange(0, HD, 2), np.arange(1, HD, 2)])
    in_maps = []
    for c in range(NCORES):
        rows = slice(c * OSL, (c + 1) * OSL)
        wq_c = np.asarray(wq)[rows].reshape(HPC, HD, D)[:, perm, :].reshape(OSL, D)
        wk_c = np.asarray(wk)[rows].reshape(HPC, HD, D)[:, perm, :].reshape(OSL, D)
        in_maps.append({
            "xq": xq,
            "wqT": _block_w(np.ascontiguousarray(wq_c.T)),
            "wkT": _block_w(np.ascontiguousarray(wk_c.T)),
            "wvT": _block_w(np.ascontiguousarray(np.asarray(wv)[rows].T)),
            "woT": _block_w(np.ascontiguousarray(np.asarray(wo)[rows].T)),
            "cosd": cosd,
            "sind": sind,
        })
    return in_maps


def run(inputs, trace=False, trace_cores=None):
    """Build (cached), run on 8 cores; returns (full_output, BassKernelResults)."""
    global _NC_CACHE
    from concourse.bass_utils import run_bass_kernel_spmd
    if _NC_CACHE is None:
        _NC_CACHE = _build()
    in_maps = _shard_inputs(**inputs)
    res = run_bass_kernel_spmd(
        _NC_CACHE, in_maps, core_ids=list(range(NCORES)), trace=trace,
        trace_cores=trace_cores)
    parts = [np.ascontiguousarray(
        np.asarray(res.results[c]["out"]).astype(np.float32).T)
        for c in range(NCORES)]
    full = np.concatenate(parts, axis=1).reshape(B, S, D)
    return full, res


def kernel(x, freqs_cos, freqs_sin, wq, wk, wv, wo):
    full, _ = run(dict(x=x, freqs_cos=freqs_cos, freqs_sin=freqs_sin,
                       wq=wq, wk=wk, wv=wv, wo=wo))
    return full
